# revision 27
# baseline (speedup 1.0000x reference)
"""Trainium2 Bass kernel for nn_EvacPolicy (segment_reduce).

Data-parallel over 8 NeuronCores: nodes sharded at graph boundaries, MLP
weights replicated, per-graph segment mean computed locally per shard,
heads computed locally per shard (row-wise independent), host concatenates
per-core outputs.

Segment mean strategy: graphs are sorted by (max-over-cores) size and packed
into runs of equal padded width s; every core places graph-at-position-p in
the SAME column range, so one SPMD program serves all 8 cores.  A single
DVE tensor_reduce over a [P, k, s] view then produces k graph sums at once.
The inf-branch stream stacks two graphs per column range (partitions 0:64 /
64:128), halving its reduce work.  All offsets are baked at trace time; the
program is rebuilt per kernel() call, nothing input-specific lives in this
file.

Engine budget: PE does the four per-node matmuls in bf16; ACT does the two
gelu layers fused with PSUM evacuation (optionally a few blocks' outer gelu
runs as an exact small-|u| quadratic on DVE to rebalance); DVE does the
grouped segment reduces.
"""

import math
import os
import sys
from contextlib import ExitStack

try:
    import concourse  # noqa: F401  (already on path, e.g. axon site)
except ImportError:
    for _p in ("/opt/trn_rl_repo",):
        if _p not in sys.path and os.path.isdir(_p):
            sys.path.insert(0, _p)

import numpy as np

import concourse.bass as bass
import concourse.bacc as bacc
import concourse.tile as tile
import concourse.mybir as mybir
from concourse.bass_utils import run_bass_kernel_spmd

FP32 = mybir.dt.float32
BF16 = mybir.dt.bfloat16
GELU = mybir.ActivationFunctionType.Gelu
IDENT = mybir.ActivationFunctionType.Identity
ADD = mybir.AluOpType.add
SUB = mybir.AluOpType.subtract
MUL = mybir.AluOpType.mult
AXX = mybir.AxisListType.X

N_CORES = 8
UNIT = 1024          # node columns per PSUM block (2 banks fp32)
MACRO = 4096         # node columns per SBUF feature tile (4 PSUM blocks)
MMN = 512            # max moving free dim per matmul into one PSUM bank
KMAX = 32            # max graphs per grouped reduce
GELU_C2 = 0.3989422804014327 / 2.0  # gelu(u) ~ u*(0.5 + GELU_C2*u), |u|<<1


def _round_up(x, m):
    return (x + m - 1) // m * m


def const_layout(G_PAD):
    return [
        ("b1a", 128, 1), ("b2a", 128, 1), ("b1i", 128, 1), ("b2i", 128, 1),
        ("pfa", 1, 128), ("pfb", 1, 64), ("ones", 1, 128),
        ("fc1w", 128, 256), ("fc1wb", 64, 256),
        ("fc1b0", 128, 1), ("fc1b1", 128, 1),
        ("fc2w0", 128, 128), ("fc2w1", 128, 128), ("fc2b", 128, 1),
        ("shgdw", 128, 2), ("shgdb", 2, 1),
        ("c1w", 128, 128), ("c1wb", 64, 128), ("c1b", 128, 1),
        ("c2w", 128, 64), ("c2b", 64, 1), ("c3w", 64, 1), ("c3b", 1, 1),
        ("recip", 1, G_PAD), ("npadA", 1, G_PAD), ("npadB", 1, G_PAD),
    ]


def pack_runs(widths, kmax=KMAX, macro=MACRO):
    """Pack (position-ordered, non-increasing) widths into runs.

    Returns (runs, col0, total_cols): runs = list of
    (macro_idx, src_off, k, s, pos0); col0[p] = start column of position p.
    """
    runs = []
    col0 = np.zeros(len(widths), np.int64)
    mac = 0
    mac_used = 0
    p = 0
    n = len(widths)
    while p < n:
        s = int(widths[p])
        assert s <= macro, f"graph width {s} exceeds macro tile {macro}"
        if mac_used + s > macro:
            mac += 1
            mac_used = 0
        k = 1
        while (p + k < n and k < kmax and widths[p + k] >= 0
               and mac_used + (k + 1) * s <= macro):
            k += 1
        for i in range(k):
            col0[p + i] = mac * macro + mac_used + i * s
        runs.append((mac, mac_used, k, s, p))
        mac_used += k * s
        p += k
    return runs, col0, (mac + 1) * macro


# ----------------------------------------------------------------------------
# device program
# ----------------------------------------------------------------------------

def build_program(NCA, NHB, G_PAD, GS, runs_a, runs_b, n_poly=0):
    nc = bacc.Bacc("TRN2", target_bir_lowering=False, debug=False,
                   num_devices=N_CORES)

    layout = const_layout(G_PAD)
    WCOLS = sum(c for _, _, c in layout)

    xph = nc.dram_tensor("xph", [8, NCA], BF16, kind="ExternalInput")
    xinf = nc.dram_tensor("xinf", [6, NHB], BF16, kind="ExternalInput")
    wblob_d = nc.dram_tensor("wblob", [128, WCOLS], FP32,
                             kind="ExternalInput")
    wb16_d = nc.dram_tensor("wb16", [128, 512], BF16, kind="ExternalInput")
    o_sg = nc.dram_tensor("o_sg", [2, G_PAD], FP32, kind="ExternalOutput")
    o_v = nc.dram_tensor("o_v", [1, G_PAD], FP32, kind="ExternalOutput")

    NMA = NCA // MACRO
    NMB = NHB // MACRO
    runs_a_by_mac = [[] for _ in range(NMA)]
    for mac, off, k, s, pos0 in runs_a:
        runs_a_by_mac[mac].append((off, k, s, pos0))
    runs_b_by_mac = [[] for _ in range(NMB)]
    for mac, off, k, s, pos0 in runs_b:
        runs_b_by_mac[mac].append((off, k, s, pos0))

    with tile.TileContext(nc) as tc, ExitStack() as ctx:
        const = ctx.enter_context(tc.tile_pool(name="const", bufs=1))
        stage = ctx.enter_context(tc.tile_pool(name="stage", bufs=1))

        wblob = const.tile([128, WCOLS], FP32, name="wblob")
        nc.sync.dma_start(wblob[:, :], wblob_d[:, :])
        W = {}
        off = 0
        for name, rows, cols in layout:
            W[name] = wblob[0:rows, off:off + cols]
            off += cols
        wb16 = const.tile([128, 512], BF16, name="wb16")
        nc.sync.dma_start(wb16[:, :], wb16_d[:, :])
        W["w1a"] = wb16[0:8, 0:128]
        W["w2ph"] = wb16[0:128, 128:256]
        W["w1i"] = wb16[0:6, 256:384]
        W["w2i"] = wb16[0:128, 384:512]

        suma = stage.tile([128, G_PAD], FP32, name="suma")
        sumbr = stage.tile([128, GS], FP32, name="sumbr")
        sumb = stage.tile([64, G_PAD], FP32, name="sumb")

        tc.strict_bb_all_engine_barrier()

        # ---------------- node pipeline ----------------
        with tc.tile_pool(name="xin", bufs=3) as xin, \
             tc.tile_pool(name="mid", bufs=3) as mid, \
             tc.tile_pool(name="scn", bufs=2) as scn, \
             tc.tile_pool(name="poly", bufs=2) as ptmp, \
             tc.tile_pool(name="psum", bufs=2, space="PSUM") as psum:

            poly_left = [n_poly]

            def outer_evac(ps2, sa_t, c0, bias, branch):
                """outer gelu PSUM block -> sa_t[:, c0:c0+UNIT]"""
                dst = sa_t[:, c0:c0 + UNIT]
                if poly_left[0] > 0:
                    poly_left[0] -= 1
                    t1 = ptmp.tile([128, UNIT], FP32, name="pt", tag="pt")
                    nc.vector.tensor_scalar(t1[:, :], ps2[:, :], GELU_C2,
                                            0.5, MUL, ADD)
                    nc.vector.tensor_tensor(dst, t1[:, :], ps2[:, :], MUL)
                else:
                    nc.scalar.activation(dst, ps2[:, :], GELU, bias=bias)

            def mlp_block(xt_src, u, w1, w2, b1, b2, sa_t, c0, tagp):
                """one UNIT of nodes: x -> gelu -> L2 -> outer -> sa_t"""
                xt = xin.tile(list(xt_src[0]), BF16, name="xt" + tagp,
                              tag="xt" + tagp)
                nc.sync.dma_start(xt[:, :], xt_src[1])
                ps1 = psum.tile([128, UNIT], FP32, name="p1" + tagp,
                                tag="l1")
                for kk in range(UNIT // MMN):
                    s = slice(kk * MMN, (kk + 1) * MMN)
                    nc.tensor.matmul(ps1[:, s], w1, xt[:, s],
                                     start=True, stop=True)
                h1 = mid.tile([128, UNIT], BF16, name="h1" + tagp,
                              tag="h1" + tagp)
                nc.scalar.activation(h1[:, :], ps1[:, :], GELU, bias=b1)
                ps2 = psum.tile([128, UNIT], FP32, name="p2" + tagp,
                                tag="l2")
                for kk in range(UNIT // MMN):
                    s = slice(kk * MMN, (kk + 1) * MMN)
                    nc.tensor.matmul(ps2[:, s], w2, h1[:, s],
                                     start=True, stop=True)
                outer_evac(ps2, sa_t, c0, b2, tagp)

            def reduce_macro(sa_t, rlist, out_t, out_map):
                for off, k, s, pos0 in rlist:
                    src = sa_t[:, off:off + k * s]
                    if k > 1:
                        src = src.rearrange("p (k s) -> p k s", k=k)
                    else:
                        src = sa_t[:, off:off + s].rearrange(
                            "p (k s) -> p k s", k=1)
                    o0 = out_map(pos0)
                    nc.vector.tensor_reduce(out_t[:, o0:o0 + k], src,
                                            AXX, ADD)

            # flatten into blocks, software-pipelined so each block's L1
            # matmul is emitted before the previous block's ACT-dependent
            # stages — keeps the PE queue full (back-to-back matmuls).
            blocks = []          # (stream, macro, j)
            bi = 0
            for m in range(NMA):
                for j in range(MACRO // UNIT):
                    blocks.append(("A", m, j))
                if m % 2 == 1 and bi < NMB:
                    for j in range(MACRO // UNIT):
                        blocks.append(("B", bi, j))
                    bi += 1
            while bi < NMB:
                for j in range(MACRO // UNIT):
                    blocks.append(("B", bi, j))
                bi += 1

            sa_cur = {"A": None, "B": None}
            state = {}

            def stage_front(b):
                stream, m, j = b
                c = (m * (MACRO // UNIT) + j) * UNIT
                if j == 0:
                    sa_cur[stream] = scn.tile([128, MACRO], FP32,
                                              name="sa" + stream,
                                              tag="sa" + stream)
                if stream == "A":
                    src = xph[:, c:c + UNIT]
                    rows, w1 = 8, W["w1a"]
                else:
                    src = xinf[:, c:c + UNIT]
                    rows, w1 = 6, W["w1i"]
                xt = xin.tile([rows, UNIT], BF16, name="xt" + stream,
                              tag="xt" + stream)
                nc.sync.dma_start(xt[:, :], src)
                ps1 = psum.tile([128, UNIT], FP32, name="p1" + stream,
                                tag="l1")
                for kk in range(UNIT // MMN):
                    s = slice(kk * MMN, (kk + 1) * MMN)
                    nc.tensor.matmul(ps1[:, s], w1, xt[:, s],
                                     start=True, stop=True)
                state[b] = (ps1, sa_cur[stream])

            def stage_back(b):
                stream, m, j = b
                ps1, sa_t = state.pop(b)
                if stream == "A":
                    w2, b1, b2 = W["w2ph"], W["b1a"], W["b2a"]
                    rlist, out_t = runs_a_by_mac[m], suma
                else:
                    w2, b1, b2 = W["w2i"], W["b1i"], W["b2i"]
                    rlist, out_t = runs_b_by_mac[m], sumbr
                h1 = mid.tile([128, UNIT], BF16, name="h1" + stream,
                              tag="h1" + stream)
                nc.scalar.activation(h1[:, :], ps1[:, :], GELU, bias=b1)
                ps2 = psum.tile([128, UNIT], FP32, name="p2" + stream,
                                tag="l2")
                for kk in range(UNIT // MMN):
                    s = slice(kk * MMN, (kk + 1) * MMN)
                    nc.tensor.matmul(ps2[:, s], w2, h1[:, s],
                                     start=True, stop=True)
                outer_evac(ps2, sa_t, j * UNIT, b2, stream)
                if j == MACRO // UNIT - 1:
                    reduce_macro(sa_t, rlist, out_t, lambda p: p)

            for i, b in enumerate(blocks):
                stage_front(b)
                if i >= 1:
                    stage_back(blocks[i - 1])
            stage_back(blocks[-1])

        # ---------------- assemble per-graph sums ----------------
        tc.strict_bb_all_engine_barrier()
        # half-0 sums sit on partitions 0:64 at positions 0..GS-1
        nc.vector.tensor_copy(sumb[:, 0:GS], sumbr[0:64, :])
        # half-1 sums: partitions 64:128 -> partition shift via DMA
        nc.sync.dma_start(sumb[:, GS:G_PAD], sumbr[64:128, 0:G_PAD - GS])

        # ---------------- heads ----------------
        tc.strict_bb_all_engine_barrier()
        with tc.tile_pool(name="hps", bufs=2, space="PSUM") as hps, \
             tc.tile_pool(name="hsb", bufs=2) as hsb:

            for g0 in range(0, G_PAD, MMN):
                gs = min(MMN, G_PAD - g0)
                gsl = slice(g0, g0 + gs)

                pr = hps.tile([128, gs], FP32, name="pr", tag="hp")
                nc.tensor.matmul(pr[:, :], W["ones"], W["recip"][:, gsl],
                                 start=True, stop=True)
                pc = hps.tile([128, gs], FP32, name="pc", tag="hp")
                nc.tensor.matmul(pc[:, :], W["pfa"], W["npadA"][:, gsl],
                                 start=True, stop=True)

                gea = hsb.tile([128, gs], FP32, name="gea", tag="ga")
                nc.vector.tensor_tensor(gea[:, :], suma[:, gsl], pc[:, :],
                                        SUB)
                nc.vector.tensor_tensor(gea[:, :], gea[:, :], pr[:, :], MUL)

                pcb = hps.tile([64, gs], FP32, name="pcb", tag="hp")
                nc.tensor.matmul(pcb[:, :], W["pfb"], W["npadB"][:, gsl],
                                 start=True, stop=True)
                geb = hsb.tile([64, gs], FP32, name="geb", tag="gb")
                nc.vector.tensor_tensor(geb[:, :], sumb[:, gsl], pcb[:, :],
                                        SUB)
                nc.vector.tensor_tensor(geb[:, :], geb[:, :], pr[0:64, :],
                                        MUL)

                def lin2(lhs_a, lhs_b, bias_t, act, m0, m1, name):
                    p = hps.tile([m1 - m0, gs], FP32, name="p" + name,
                                 tag="hp")
                    nc.tensor.matmul(p[:, :], lhs_a[:, m0:m1], gea[:, :],
                                     start=True, stop=False)
                    nc.tensor.matmul(p[:, :], lhs_b[:, m0:m1], geb[:, :],
                                     start=False, stop=True)
                    o = hsb.tile([m1 - m0, gs], FP32, name="s" + name,
                                 tag="hs" + name)
                    nc.scalar.activation(o[:, :], p[:, :], act, bias=bias_t)
                    return o

                d10 = lin2(W["fc1w"], W["fc1wb"], W["fc1b0"], GELU,
                           0, 128, "d10")
                d11 = lin2(W["fc1w"], W["fc1wb"], W["fc1b1"], GELU,
                           128, 256, "d11")

                pd2 = hps.tile([128, gs], FP32, name="pd2", tag="hp")
                nc.tensor.matmul(pd2[:, :], W["fc2w0"], d10[:, :],
                                 start=True, stop=False)
                nc.tensor.matmul(pd2[:, :], W["fc2w1"], d11[:, :],
                                 start=False, stop=True)
                d2 = hsb.tile([128, gs], FP32, name="d2", tag="d2")
                nc.scalar.activation(d2[:, :], pd2[:, :], GELU,
                                     bias=W["fc2b"])

                psg = hps.tile([2, gs], FP32, name="psg", tag="hp")
                nc.tensor.matmul(psg[:, :], W["shgdw"], d2[:, :],
                                 start=True, stop=True)
                sg = hsb.tile([2, gs], FP32, name="sg", tag="sg")
                nc.scalar.activation(sg[:, :], psg[:, :], IDENT,
                                     bias=W["shgdb"])
                nc.sync.dma_start(o_sg[:, gsl], sg[:, :])

                v1 = lin2(W["c1w"], W["c1wb"], W["c1b"], GELU, 0, 128, "v1")
                pv2 = hps.tile([64, gs], FP32, name="pv2", tag="hp")
                nc.tensor.matmul(pv2[:, :], W["c2w"], v1[:, :],
                                 start=True, stop=True)
                v2 = hsb.tile([64, gs], FP32, name="v2", tag="v2")
                nc.scalar.activation(v2[:, :], pv2[:, :], GELU,
                                     bias=W["c2b"])
                pv3 = hps.tile([1, gs], FP32, name="pv3", tag="hp")
                nc.tensor.matmul(pv3[:, :], W["c3w"], v2[:, :],
                                 start=True, stop=True)
                vo = hsb.tile([1, gs], FP32, name="vo", tag="vo")
                nc.scalar.activation(vo[:, :], pv3[:, :], IDENT,
                                     bias=W["c3b"])
                nc.sync.dma_start(o_v[:, gsl], vo[:, :])

    return nc


# ----------------------------------------------------------------------------
# host wrapper
# ----------------------------------------------------------------------------

_cache = {}


def _np_gelu(x):
    v = np.vectorize(math.erf)
    return 0.5 * x * (1.0 + v(x / math.sqrt(2.0)))


def _blockdiag(blocks, K, M):
    out = np.zeros((K, M), np.float32)
    for Wm, r, c in blocks:
        out[r:r + Wm.shape[0], c:c + Wm.shape[1]] = Wm
    return out


N_POLY = int(os.environ.get("KERNEL_N_POLY", "0"))


def kernel(x_ped, x_hazard, x_infra, batch, num_graphs,
           ped_W1, ped_b1, ped_W2, ped_b2,
           haz_W1, haz_b1, haz_W2, haz_b2,
           inf_W1, inf_b1, inf_W2, inf_b2,
           fc1_W, fc1_b, fc2_W, fc2_b,
           sh_W, sh_b, gd_W, gd_b,
           c1_W, c1_b, c2_W, c2_b, c3_W, c3_b):
    import ml_dtypes
    bf16 = ml_dtypes.bfloat16

    x_ped = np.asarray(x_ped, np.float32)
    x_hazard = np.asarray(x_hazard, np.float32)
    x_infra = np.asarray(x_infra, np.float32)
    batch = np.asarray(batch).astype(np.int64)
    B = int(num_graphs)
    N = batch.shape[0]

    f32 = lambda a: np.ascontiguousarray(np.asarray(a), dtype=np.float32)
    (ped_W1, ped_b1, ped_W2, ped_b2, haz_W1, haz_b1, haz_W2, haz_b2,
     inf_W1, inf_b1, inf_W2, inf_b2, fc1_W, fc1_b, fc2_W, fc2_b,
     sh_W, sh_b, gd_W, gd_b, c1_W, c1_b, c2_W, c2_b, c3_W, c3_b) = map(
        f32, (ped_W1, ped_b1, ped_W2, ped_b2, haz_W1, haz_b1, haz_W2,
              haz_b2, inf_W1, inf_b1, inf_W2, inf_b2, fc1_W, fc1_b, fc2_W,
              fc2_b, sh_W, sh_b, gd_W, gd_b, c1_W, c1_b, c2_W, c2_b, c3_W,
              c3_b))

    # ---- shard graphs across cores, balancing node counts ----
    ends = np.searchsorted(batch, np.arange(B), side="right")
    gsplits = [0]
    for c in range(1, N_CORES):
        gsplits.append(int(np.searchsorted(ends, N * c // N_CORES)))
    gsplits.append(B)
    g_lo = np.array(gsplits[:-1])
    g_hi = np.maximum(np.array(gsplits[1:]), g_lo)

    counts_all = np.diff(np.concatenate([[0], ends])).astype(np.int64)
    G_PAD = _round_up(max(int((g_hi - g_lo).max()), 2), 64)

    cnt = np.zeros((N_CORES, G_PAD), np.int64)
    for c in range(N_CORES):
        g = g_hi[c] - g_lo[c]
        cnt[c, :g] = counts_all[g_lo[c]:g_hi[c]]
    maxcnt = np.maximum(cnt.max(axis=0), 1)

    # ---- stream positions: rank graphs by width, interleave halves ----
    order = np.argsort(-maxcnt, kind="stable")       # local idx by rank
    GS = (G_PAD + 1) // 2
    pos_of = np.zeros(G_PAD, np.int64)               # local idx -> position
    for r, j in enumerate(order):
        q, h = divmod(r, 2)
        pos_of[j] = q if h == 0 else GS + q
    # guard: ranks 2q/2q+1 -> positions q / GS+q; q < GS always
    idx_at = np.zeros(G_PAD, np.int64)               # position -> local idx
    idx_at[pos_of] = np.arange(G_PAD)

    wA = maxcnt[idx_at]                              # width by position
    # A-stream: positions 0..GS-1 then GS..: widths non-increasing within
    # each half (sorted interleave), pack runs per half consecutively
    runs_a0, col0_a0, colsA0 = pack_runs(wA[:GS])
    runs_a1, col0_a1, colsA1 = pack_runs(wA[GS:])
    off_macs = colsA0 // MACRO
    runs_a = runs_a0 + [(m + off_macs, o, k, s, p + GS)
                        for m, o, k, s, p in runs_a1]
    col0A = np.concatenate([col0_a0, col0_a1 + colsA0])
    NCA = colsA0 + colsA1

    # B-stream: pair (pos q, pos GS+q) share a column range
    wB = np.maximum(wA[:GS], np.concatenate(
        [wA[GS:], np.ones(2 * GS - G_PAD, np.int64)]))
    runs_b, col0B_pair, NHB = pack_runs(wB)
    col0B = np.concatenate([col0B_pair, col0B_pair[:G_PAD - GS]])

    # ---- per-core node arrays ----
    starts_all = ends - counts_all
    x_all = np.concatenate([x_ped, x_hazard, x_infra], axis=1)

    in_maps = []
    for c in range(N_CORES):
        n0 = int(ends[g_lo[c] - 1]) if g_lo[c] > 0 else 0
        n1 = int(ends[g_hi[c] - 1]) if g_hi[c] > 0 else 0
        ncr = n1 - n0
        g = batch[n0:n1] - g_lo[c]                   # local graph idx
        within = np.arange(ncr) - (starts_all[batch[n0:n1]] - n0)
        p = pos_of[g]
        destA = col0A[p] + within
        destB = col0B[p] + within
        hB = (p >= GS).astype(np.int64)

        xph_a = np.zeros((8, NCA), bf16)
        xph_a[:, destA] = x_all[n0:n1].T.astype(bf16)
        xinf_a = np.zeros((6, NHB), bf16)
        xinf_T = x_infra[n0:n1].T.astype(bf16)
        lo = hB == 0
        xinf_a[0:3, destB[lo]] = xinf_T[:, lo]
        xinf_a[3:6, destB[~lo]] = xinf_T[:, ~lo]
        in_maps.append({"xph": xph_a, "xinf": xinf_a})

    # feature vector produced by zero-input (padding) nodes
    pf = []
    for W1, b1, W2, b2 in ((ped_W1, ped_b1, ped_W2, ped_b2),
                           (haz_W1, haz_b1, haz_W2, haz_b2),
                           (inf_W1, inf_b1, inf_W2, inf_b2)):
        pf.append(_np_gelu(_np_gelu(b1) @ W2 + b2))
    pad_feat = np.concatenate(pf).astype(np.float32)

    consts = {
        "b1a": np.concatenate([ped_b1, haz_b1]).reshape(128, 1),
        "b2a": np.concatenate([ped_b2, haz_b2]).reshape(128, 1),
        "b1i": np.concatenate([inf_b1, inf_b1]).reshape(128, 1),
        "b2i": np.concatenate([inf_b2, inf_b2]).reshape(128, 1),
        "pfa": pad_feat[:128].reshape(1, 128),
        "pfb": pad_feat[128:].reshape(1, 64),
        "ones": np.ones((1, 128), np.float32),
        "fc1w": fc1_W[0:128], "fc1wb": fc1_W[128:192],
        "fc1b0": fc1_b[0:128].reshape(128, 1),
        "fc1b1": fc1_b[128:256].reshape(128, 1),
        "fc2w0": fc2_W[0:128], "fc2w1": fc2_W[128:256],
        "fc2b": fc2_b.reshape(128, 1),
        "shgdw": np.concatenate([sh_W, gd_W], axis=1),
        "shgdb": np.array([[float(sh_b.ravel()[0])],
                           [float(gd_b.ravel()[0])]], np.float32),
        "c1w": c1_W[0:128], "c1wb": c1_W[128:192],
        "c1b": c1_b.reshape(128, 1),
        "c2w": c2_W, "c2b": c2_b.reshape(64, 1),
        "c3w": c3_W, "c3b": c3_b.reshape(1, 1),
    }

    layout = const_layout(G_PAD)
    WCOLS = sum(c for _, _, c in layout)
    blob_common = np.zeros((128, WCOLS), np.float32)
    slices = {}
    off = 0
    for name, rows, cols in layout:
        slices[name] = (rows, slice(off, off + cols))
        if name in consts:
            blob_common[0:rows, off:off + cols] = consts[name]
        off += cols

    wb16 = np.zeros((128, 512), bf16)
    wb16[0:8, 0:128] = _blockdiag(
        [(ped_W1, 0, 0), (haz_W1, 2, 64)], 8, 128).astype(bf16)
    wb16[0:128, 128:256] = _blockdiag(
        [(ped_W2, 0, 0), (haz_W2, 64, 64)], 128, 128).astype(bf16)
    wb16[0:6, 256:384] = _blockdiag(
        [(inf_W1, 0, 0), (inf_W1, 3, 64)], 6, 128).astype(bf16)
    wb16[0:128, 384:512] = _blockdiag(
        [(inf_W2, 0, 0), (inf_W2, 64, 64)], 128, 128).astype(bf16)

    cnt_pos = cnt[:, idx_at]                         # (cores, positions)
    sA = wA                                          # padded width (A)
    sB = np.concatenate([wB, wB[:G_PAD - GS]])       # padded width (B)
    for c in range(N_CORES):
        blob = blob_common.copy()
        rows, sl = slices["recip"]
        blob[0:rows, sl] = (1.0 / np.maximum(cnt_pos[c], 1)).astype(
            np.float32)
        rows, sl = slices["npadA"]
        blob[0:rows, sl] = (sA - cnt_pos[c]).astype(np.float32)
        rows, sl = slices["npadB"]
        blob[0:rows, sl] = (sB - cnt_pos[c]).astype(np.float32)
        in_maps[c]["wblob"] = blob
        in_maps[c]["wb16"] = wb16

    # ---- build / fetch program ----
    # the quadratic outer-gelu path assumes zero second-layer biases
    n_poly = N_POLY
    if (np.any(ped_b2) or np.any(haz_b2) or np.any(inf_b2)):
        n_poly = 0
    key = (NCA, NHB, G_PAD, GS, tuple(map(tuple, runs_a)),
           tuple(map(tuple, runs_b)), n_poly)
    if key not in _cache:
        _cache.clear()
        nc_new = build_program(NCA, NHB, G_PAD, GS, runs_a, runs_b,
                               n_poly=n_poly)
        if not nc_new.is_finalized():
            nc_new.finalize()
        _cache[key] = nc_new
    nc = _cache[key]

    trace = bool(os.environ.get("BASS_PROFILE"))
    if trace:
        _ensure_ntff_hook()
    res = run_bass_kernel_spmd(nc, in_maps, list(range(N_CORES)),
                               trace=trace)
    if trace and res.exec_time_ns is not None:
        kernel.last_exec_time_ns = res.exec_time_ns
        kernel.last_result = res
        print(f"HW exec time: {res.exec_time_ns} ns")

    # ---- assemble full outputs (positions -> graph ids) ----
    shelter = np.zeros((B, 1), np.float32)
    guidance = np.zeros((B, 1), np.float32)
    value = np.zeros((B,), np.float32)
    for c in range(N_CORES):
        g = g_hi[c] - g_lo[c]
        if g == 0:
            continue
        out_sg = res.results[c]["o_sg"]
        out_v = res.results[c]["o_v"]
        pg = pos_of[np.arange(g)]
        shelter[g_lo[c]:g_hi[c], 0] = out_sg[0, pg]
        guidance[g_lo[c]:g_hi[c], 0] = out_sg[1, pg]
        value[g_lo[c]:g_hi[c]] = out_v[0, pg]
    return shelter, guidance, value


kernel.last_exec_time_ns = None
kernel.last_result = None


def _ensure_ntff_hook():
    """The agent image lacks ``antenv.axon_hooks``; shim it with the
    ctypes NTFF profiler from trn_agent_boot so trace=True works."""
    import types
    try:
        from antenv.axon_hooks import get_axon_ntff_profile_hook  # noqa
        return
    except ImportError:
        pass
    try:
        sys.path.insert(0, "/root/.axon_site")
        from trn_agent_boot.trn_boot import _ntff_profile_via_ctypes
        hook = _ntff_profile_via_ctypes("/opt/axon/libaxon_pjrt.so")
    except Exception:
        hook = None
    store = {"h": hook}
    pkg = sys.modules.setdefault("antenv", types.ModuleType("antenv"))
    mod = types.ModuleType("antenv.axon_hooks")
    mod.get_axon_ntff_profile_hook = lambda: store["h"]
    mod.set_axon_ntff_profile_hook = lambda h: store.update(h=h)
    pkg.axon_hooks = mod
    sys.modules["antenv.axon_hooks"] = mod
